# revision 17
# baseline (speedup 1.0000x reference)
"""Causal multi-head attention (B=2, T=2048, D=1024, H=16) on 8 TRN2 NeuronCores.

Sharding: core c owns heads {2c, 2c+1} (= 128 contiguous dims of D) of BOTH
batches — head-parallel over all 8 cores, batch handled inside each core.
This makes the output-projection exchange a single 8-core AllToAll per q-span
of the (normalized, bf16) attention outputs: shard j of core c's send buffer
is its yT slice for (batch j//4, q-tile j%4), and received slot i is D-chunk
i for the core's own (batch, q-tile) = (c//4, c%4). Every AP in that exchange
is core-independent, so one SPMD program serves all 8 cores, and the wire
traffic is ~1MB bf16 total instead of ReduceScattering 8MB of fp32 partials
per core. Each core then computes the full-D out-projection for its q-tile.

Device-side layout (host pre-transposes, pure data movement):
  - xT  [2, D, T]     = x[b].T so projections contract D on the partition dim.
  - qT/kT [b][128, T] computed directly transposed (dims on partitions);
                        the core's 2 heads at partitions 0-63 / 64-127.
  - scoresT[k, q]     = k @ qT; the two heads are computed by two row-tiled
                        matmuls (tile_position (0,0)/(64,0), K=64 each) that
                        run concurrently in the PE array, writing two
                        adjacent PSUM banks.
  - exp               one ScalarE activation per k-tile covers both heads'
                        scores ([128, 1024] across the 2 banks). Diagonal
                        tiles trim the leading fully-masked columns from the
                        scores matmul, the exp, and the AV matmul; the mask
                        values are applied only on the [128, 128] triangle
                        blocks.
  - v_aug [k, 2*65]   v with a ones column per head: AV yields yT' [65, span]
                        whose row 64 is the softmax denominator.
  - normalization     reciprocal of the denominator rows, broadcast across
                        partitions with one rank-33 selector matmul per
                        (span, batch), multiplied into yT in one DVE pass.
  - out-projection    after the AllToAll: 8 accumulating matmuls per
                        [128 q, 512] output tile (full-D contraction), bias
                        on DVE, DMA straight to the output.

Dtypes: all matmul operands bf16 with fp32 PSUM accumulation; exp and the
normalization run in fp32 (bf16 storage). ScalarE does nothing but exp; the
PE is kept warm with a short warm-up matmul burst and by interleaving
projection / out-projection matmuls between attention blocks.
"""

import os
import numpy as np
import ml_dtypes

BF16 = ml_dtypes.bfloat16

B, T, D, H = 2, 2048, 1024, 16
HD = D // H                     # 64
NCORES = 8
DL = D // NCORES                # dims per core = 128 (2 heads)
SP = 512                        # free-dim span per matmul (one PSUM bank, fp32)
QS = T // SP                    # 4 q spans
KT = T // 128                   # 16 k tiles
SCALE = HD ** -0.5

_CACHE = {}


def _build_program():
    import concourse.bass as bass  # noqa: F401  (registers bass machinery)
    import concourse.tile as tile
    from concourse import bacc, mybir

    f32 = mybir.dt.float32
    f32r = mybir.dt.float32r
    bf16 = mybir.dt.bfloat16
    Exp = mybir.ActivationFunctionType.Exp

    nc = bacc.Bacc("TRN2", target_bir_lowering=False, debug=False,
                   num_devices=NCORES)

    xT = nc.dram_tensor("xT", [B, D, T], bf16, kind="ExternalInput")
    wqT = nc.dram_tensor("wqT", [D, DL], bf16, kind="ExternalInput")
    wkT = nc.dram_tensor("wkT", [D, DL], bf16, kind="ExternalInput")
    wvT = nc.dram_tensor("wvT", [D, DL], bf16, kind="ExternalInput")
    woT = nc.dram_tensor("woT", [D, D], bf16, kind="ExternalInput")
    bqP = nc.dram_tensor("bqP", [128, 1], f32, kind="ExternalInput")
    bkP = nc.dram_tensor("bkP", [128, 1], f32, kind="ExternalInput")
    bv = nc.dram_tensor("bv", [1, DL], bf16, kind="ExternalInput")
    bo = nc.dram_tensor("bo", [1, D], bf16, kind="ExternalInput")
    mtriD = nc.dram_tensor("mtriD", [128, B * KT * 128], bf16,
                           kind="ExternalInput")
    out_ext = nc.dram_tensor("out", [QS, 128, D], f32, kind="ExternalOutput")

    RG = [[0, 1, 2, 3, 4, 5, 6, 7]]

    with tile.TileContext(nc) as tc:
        with tc.tile_pool(name="main", bufs=1) as main, \
             tc.tile_pool(name="dram", bufs=1, space="DRAM") as dram:
            xt_s = main.tile([128, B, 8, T], bf16)
            wq_s = main.tile([128, 8, DL], bf16)
            wk_s = main.tile([128, 8, DL], bf16)
            wv_s = main.tile([128, 8, DL], bf16)
            woT_s = main.tile([128, 8, D], bf16)
            qT_s = main.tile([128, B, T], bf16)
            kT_s = main.tile([128, B, T], bf16)
            yT_s = main.tile([128, B, T], bf16)
            v_s = main.tile([128, B, KT, 2 * 65], bf16)
            bq_s = main.tile([128, 1], f32)
            bk_s = main.tile([128, 1], f32)
            bv_bc = main.tile([128, DL], bf16)
            bo_bc = main.tile([128, D], bf16)
            mtri_s = main.tile([128, B, KT, 128], bf16)
            # selector for the denominator broadcast: rb = sel.T @ rec2
            # (rec2 rows 0/32 hold the two heads' 1/denominator; the other
            # rows are 1.0 and get selected by zeros)
            sel_s = main.tile([33, 128], bf16)
            rec_all = main.tile([33, B * QS, SP], bf16)
            recf_all = main.tile([33, B * QS, SP], f32)
            den_all = main.tile([33, B * QS, SP], f32)
            warm_s = main.tile([128, SP], bf16)
            dum_o = main.tile([1, 2], bf16)

            a2a_in = [dram.tile([NCORES * 128, 128], bf16, name=f"a2ai{s}")
                      for s in range(QS)]
            a2a_out = [dram.tile([NCORES * 128, 128], bf16, name=f"a2ao{s}")
                       for s in range(QS)]
            # constants (DVE) + ACT table warm-up before any real dependency
            nc.vector.memset(warm_s, 0.25)
            nc.vector.memset(v_s, 1.0)
            nc.vector.memset(sel_s, 0.0)
            nc.vector.memset(sel_s[0:1, 0:64], 1.0)
            nc.vector.memset(sel_s[32:33, 64:128], 1.0)
            nc.vector.memset(rec_all, 1.0)
            nc.vector.memset(recf_all, 1.0)
            nc.vector.memset(den_all, 1.0)
            nc.scalar.activation(dum_o, warm_s[0:1, 0:2], Exp)

            # loads: wq + batch-0 x first (the first projections need
            # them), mask triangles as one contiguous DMA, then the rest
            nc.sync.dma_start(out=bq_s, in_=bqP[:])
            nc.sync.dma_start(out=bk_s, in_=bkP[:])
            wq_r = wqT[:].rearrange("(c p) n -> c p n", p=128)
            for c in range(8):
                nc.sync.dma_start(out=wq_s[:, c, :], in_=wq_r[c])
            xT_r = xT[:].rearrange("b (c p) t -> b c p t", p=128)
            for c in range(8):
                eng = nc.sync if c % 2 == 0 else nc.gpsimd
                eng.dma_start(out=xt_s[:, 0, c, :], in_=xT_r[0, c])
            for w_s, w_d in ((wk_s, wkT), (wv_s, wvT)):
                w_r = w_d[:].rearrange("(c p) n -> c p n", p=128)
                for c in range(8):
                    nc.gpsimd.dma_start(out=w_s[:, c, :], in_=w_r[c])
            for c in range(8):
                eng = nc.sync if c % 2 == 0 else nc.gpsimd
                eng.dma_start(out=xt_s[:, 1, c, :], in_=xT_r[1, c])
            nc.sync.dma_start(
                out=mtri_s[:].rearrange("p b t q -> p (b t q)"),
                in_=mtriD[:])
            nc.gpsimd.dma_start(out=bv_bc, in_=bv[:].to_broadcast([128, DL]))
            nc.gpsimd.dma_start(out=bo_bc, in_=bo[:].to_broadcast([128, D]))
            woT_r = woT[:].rearrange("(c p) n -> c p n", p=128)
            for c in range(8):
                nc.gpsimd.dma_start(out=woT_s[:, c, :], in_=woT_r[c])

            with tc.tile_pool(name="sc_psum", bufs=2, space="PSUM") as sc_psum, \
                 tc.tile_pool(name="av_psum", bufs=1, space="PSUM") as av_psum, \
                 tc.tile_pool(name="mm_psum", bufs=2, space="PSUM") as mm_psum, \
                 tc.tile_pool(name="at_sb", bufs=6) as at_sb, \
                 tc.tile_pool(name="ytf_sb", bufs=2) as ytf_sb, \
                 tc.tile_pool(name="ob_sb", bufs=3) as ob_sb:

                # PE warm-up during the initial DMA wait: gets the HAM clock
                # gate to 8/8 before the first projection matmul
                for i in range(16):
                    wm = mm_psum.tile([128, SP], f32, tag="mm")
                    nc.tensor.matmul(wm, lhsT=warm_s[:, 0:128], rhs=warm_s,
                                     start=True, stop=True)

                def proj_block(sp):
                    # q/k for span sp and v for k-tiles 4sp..4sp+3, per batch
                    for b in range(B):
                        for w_s, b_s, dst in ((wq_s, bq_s, qT_s),
                                              (wk_s, bk_s, kT_s)):
                            ps = mm_psum.tile([128, SP], f32, tag="mm")
                            for kc in range(8):
                                nc.tensor.matmul(
                                    ps,
                                    lhsT=w_s[:, kc, :],
                                    rhs=xt_s[:, b, kc, sp * SP:(sp + 1) * SP],
                                    start=(kc == 0), stop=(kc == 7))
                            nc.vector.tensor_scalar_add(
                                dst[:, b, sp * SP:(sp + 1) * SP], ps, b_s)
                        for mt in range(4 * sp, 4 * sp + 4):
                            ps = mm_psum.tile([128, SP], f32, tag="mm")
                            for kc in range(8):
                                nc.tensor.matmul(
                                    ps[:, 0:DL],
                                    lhsT=xt_s[:, b, kc,
                                              mt * 128:(mt + 1) * 128],
                                    rhs=wv_s[:, kc, :],
                                    start=(kc == 0), stop=(kc == 7))
                            nc.vector.tensor_add(
                                v_s[:, b, mt, :].rearrange(
                                    "p (h d) -> p h d", d=65)[:, :, 0:64],
                                ps[:, 0:DL].rearrange(
                                    "p (h d) -> p h d", d=64),
                                bv_bc.rearrange("p (h d) -> p h d", d=64))

                def attn(sp, b):
                    # both heads for batch b; returns the rec slot
                    nkt = 4 * sp + 4
                    av = av_psum.tile([65, 2 * SP], f32, tag="av")
                    for kt in range(nkt):
                        c0 = max(0, 128 * (kt - 4 * sp))
                        sc = sc_psum.tile([128, 2 * SP], f32, tag="sc")
                        for hh in range(2):
                            r0 = 64 * hh
                            nc.tensor.matmul(
                                sc[:, hh * SP + c0:(hh + 1) * SP],
                                lhsT=kT_s[r0:r0 + 64, b,
                                          kt * 128:(kt + 1) * 128],
                                rhs=qT_s[r0:r0 + 64, b,
                                         sp * SP + c0:(sp + 1) * SP],
                                start=True, stop=True)
                        at = at_sb.tile([128, 2 * SP], bf16, tag="at")
                        if c0:
                            nc.scalar.activation(
                                at.rearrange("p (g q) -> p g q",
                                             g=2)[:, :, c0:],
                                sc.rearrange("p (g q) -> p g q",
                                             g=2)[:, :, c0:],
                                Exp)
                        else:
                            nc.scalar.activation(at, sc, Exp)
                        if kt >= 4 * sp:  # diagonal tile: mask the triangle
                            for hh in range(2):
                                blk = at[:, hh * SP + c0:hh * SP + c0 + 128]
                                nc.vector.tensor_mul(blk, blk,
                                                     mtri_s[:, b, kt, :])
                        for hh in range(2):
                            nc.tensor.matmul(
                                av[:, hh * SP + c0:(hh + 1) * SP],
                                lhsT=v_s[:, b, kt, hh * 65:(hh + 1) * 65],
                                rhs=at[:, hh * SP + c0:(hh + 1) * SP],
                                start=(kt == 0), stop=(kt == nkt - 1))
                    rec2 = rec_all[:, B * sp + b, :]
                    recf = recf_all[:, B * sp + b, :]
                    den2 = den_all[:, B * sp + b, :]
                    nc.vector.tensor_copy(den2[0:1, :], av[64:65, 0:SP])
                    nc.vector.tensor_copy(den2[32:33, :], av[64:65, SP:2 * SP])
                    nc.vector.reciprocal_approx_fast(out=recf, in_=den2)
                    nc.vector.tensor_copy(rec2, recf)
                    nc.vector.tensor_copy(yT_s[0:64, b, sp * SP:(sp + 1) * SP],
                                          av[0:64, 0:SP])
                    nc.vector.tensor_copy(yT_s[64:128, b,
                                               sp * SP:(sp + 1) * SP],
                                          av[0:64, SP:2 * SP])
                    return rec2

                def post(sp, b, rec2):
                    # broadcast 1/denominator across partitions via one
                    # rank-33 selector matmul, then normalize yT in place
                    rb = mm_psum.tile([128, SP], f32, tag="mm")
                    nc.tensor.matmul(rb, lhsT=sel_s, rhs=rec2,
                                     start=True, stop=True)
                    yv = yT_s[:, b, sp * SP:(sp + 1) * SP]
                    nc.vector.tensor_mul(yv, yv, rb)

                def exchange(sp):
                    # shard j = my yT slice for (batch j//4, q-tile j%4);
                    # slot i of the output = D-chunk i of my own q-tile
                    for b in range(B):
                        for t in range(QS):
                            j = QS * b + t
                            nc.sync.dma_start(
                                out=a2a_in[sp][j * 128:(j + 1) * 128, :],
                                in_=yT_s[:, b, sp * SP + t * 128:
                                         sp * SP + (t + 1) * 128])
                    nc.gpsimd.collective_compute(
                        "AllToAll", mybir.AluOpType.bypass,
                        replica_groups=RG,
                        ins=[a2a_in[sp][:].opt()],
                        outs=[a2a_out[sp][:].opt()])
                    ytf = ytf_sb.tile([128, 8, 128], bf16, tag="ytf")
                    nc.sync.dma_start(
                        out=ytf,
                        in_=a2a_out[sp][:].rearrange("(i p) q -> p i q",
                                                     p=128))
                    return ytf

                def outproj(sp, ytf):
                    # full-D out-projection for this core's q-tile of span sp
                    for ns in range(2):
                        po = mm_psum.tile([128, SP], f32, tag="mm")
                        for i in range(8):
                            nc.tensor.matmul(
                                po,
                                lhsT=ytf[:, i, :],
                                rhs=woT_s[:, i, ns * SP:(ns + 1) * SP],
                                start=(i == 0), stop=(i == 7))
                        ob = ob_sb.tile([128, SP], f32, tag="ob")
                        nc.vector.tensor_add(ob, po,
                                             bo_bc[:, ns * SP:(ns + 1) * SP])
                        nc.sync.dma_start(
                            out=out_ext[sp, :, ns * SP:(ns + 1) * SP], in_=ob)

                # software pipeline: post()/exchange()/outproj() are issued
                # behind later attention blocks so their PE work (which waits
                # on DVE/collective results) never stalls the PE queue
                recs = {}
                ytfs = {}
                proj_block(0)
                recs[(0, 0)] = attn(0, 0)
                proj_block(1)
                recs[(0, 1)] = attn(0, 1)
                post(0, 0, recs[(0, 0)])
                proj_block(2)
                recs[(1, 0)] = attn(1, 0)
                post(0, 1, recs[(0, 1)])
                ytfs[0] = exchange(0)
                proj_block(3)
                recs[(1, 1)] = attn(1, 1)
                post(1, 0, recs[(1, 0)])
                recs[(2, 0)] = attn(2, 0)
                post(1, 1, recs[(1, 1)])
                ytfs[1] = exchange(1)
                recs[(2, 1)] = attn(2, 1)
                post(2, 0, recs[(2, 0)])
                recs[(3, 0)] = attn(3, 0)
                post(2, 1, recs[(2, 1)])
                ytfs[2] = exchange(2)
                outproj(0, ytfs[0])
                recs[(3, 1)] = attn(3, 1)
                post(3, 0, recs[(3, 0)])
                outproj(1, ytfs[1])
                post(3, 1, recs[(3, 1)])
                ytfs[3] = exchange(3)
                outproj(2, ytfs[2])
                outproj(3, ytfs[3])

    nc.compile()
    return nc


def _get_program():
    if "nc" not in _CACHE:
        _CACHE["nc"] = _build_program()
    return _CACHE["nc"]


def _make_in_maps(x, mask, Wq, bq, Wk, bk, Wv, bv, Wo, bo):
    x = np.asarray(x, np.float32)
    mask = np.asarray(mask, bool)
    Wq = np.asarray(Wq, np.float32)
    Wk = np.asarray(Wk, np.float32)
    Wv = np.asarray(Wv, np.float32)
    Wo = np.asarray(Wo, np.float32)
    bq = np.asarray(bq, np.float32)
    bk = np.asarray(bk, np.float32)
    bv = np.asarray(bv, np.float32)
    bo = np.asarray(bo, np.float32)

    xTd = np.ascontiguousarray(x.transpose(0, 2, 1)).astype(BF16)  # [B, D, T]
    woT = np.ascontiguousarray(Wo.T).astype(BF16)
    bo_row = bo.reshape(1, D).astype(BF16)
    # the 16 diagonal [128,128] blocks of mask[b,0].T (k on rows),
    # partition-major so the load is one contiguous DMA
    md = np.empty((B, KT, 128, 128), np.float32)
    for b in range(B):
        mT = mask[b, 0].T
        for t in range(KT):
            md[b, t] = mT[t * 128:(t + 1) * 128, t * 128:(t + 1) * 128]
    md = np.ascontiguousarray(
        md.transpose(2, 0, 1, 3)).reshape(128, B * KT * 128).astype(BF16)

    in_maps = []
    for c in range(NCORES):
        sl = slice(c * DL, (c + 1) * DL)  # dims of heads {2c, 2c+1}
        in_maps.append({
            "xT": xTd,
            "wqT": np.ascontiguousarray((Wq[sl] * SCALE).T).astype(BF16),
            "wkT": np.ascontiguousarray(Wk[sl].T).astype(BF16),
            "wvT": np.ascontiguousarray(Wv[sl].T).astype(BF16),
            "woT": woT,
            "bqP": np.ascontiguousarray((bq[sl] * SCALE).reshape(DL, 1)),
            "bkP": np.ascontiguousarray(bk[sl].reshape(DL, 1)),
            "bv": bv[sl].reshape(1, DL).astype(BF16),
            "bo": bo_row,
            "mtriD": md,
        })
    return in_maps


def _capture_profile(nc, in_maps, tmpdir):
    """Run with NTFF capture and process the profile ourselves (the stock
    trace path can't handle the duplicate-executable NTFFs the axon relay
    produces). Returns (results, exec_time_ns|None)."""
    import glob
    import json
    import re
    import subprocess
    from trn_agent_boot.trn_boot import _ntff_profile_via_ctypes
    from concourse import bass2jax

    hook = _ntff_profile_via_ctypes("/opt/axon/libaxon_pjrt.so")
    if hook is None:
        raise RuntimeError("libaxon_pjrt.so lacks NTFF profile symbols")
    os.makedirs(tmpdir, exist_ok=True)
    with hook(tmpdir, [0]):
        results = bass2jax.run_bass_via_pjrt(nc, in_maps, n_cores=NCORES)

    # group NTFF/NEFF pairs by executable id; use the newest executable
    ntffs = glob.glob(os.path.join(tmpdir, "*_body*-device*.ntff"))
    best, best_id = None, -1
    for f in ntffs:
        m = re.search(r"executable(\d+)-device000000", f)
        if m and int(m.group(1)) > best_id:
            best_id, best = int(m.group(1)), f
    if best is None:
        raise RuntimeError(f"no NTFF produced in {tmpdir}")
    neff = re.sub(r"-device\d+-execution-\d+\.ntff$", ".neff", best)
    out_json = os.path.join(tmpdir, "prof.json")
    subprocess.check_call(
        ["neuron-profile", "view", "--ignore-nc-buf-usage", "-s", best,
         "-n", neff, "--output-format=json", f"--output-file={out_json}"],
        cwd=tmpdir)
    summary = json.load(open(out_json))["summary"][0]
    return results, int(summary["total_time"] * 1e9)


def kernel(x, mask, Wq, bq, Wk, bk, Wv, bv, Wo, bo):
    from concourse import bass_utils

    in_maps = _make_in_maps(x, mask, Wq, bq, Wk, bk, Wv, bv, Wo, bo)
    nc = _get_program()

    trace = bool(int(os.environ.get("MHA_TRACE", "0")))
    tmpdir = os.environ.get("MHA_TRACE_DIR") or None
    results = None
    if trace and tmpdir:
        try:
            results, exec_ns = _capture_profile(nc, in_maps, tmpdir)
            _CACHE["last_exec_time_ns"] = exec_ns
        except Exception as e:  # profiling is best-effort
            print(f"profiling unavailable: {type(e).__name__}: {e}")
            results = None
    if results is None:
        results = bass_utils.run_bass_kernel_spmd(
            nc, in_maps, core_ids=list(range(NCORES))).results
        _CACHE.setdefault("last_exec_time_ns", None)

    out = np.empty((B, T, D), np.float32)
    for c in range(NCORES):
        b, t = divmod(c, QS)  # core c owns (batch b, q-tile t) of every span
        o = results[c]["out"]
        for sp in range(QS):
            lo = sp * SP + t * 128
            out[b, lo:lo + 128] = o[sp]
    return out


# revision 18
# speedup vs baseline: 1.1224x; 1.1224x over previous
"""Causal multi-head attention (B=2, T=2048, D=1024, H=16) on 8 TRN2 NeuronCores.

Sharding: core c owns heads {2c, 2c+1} (= 128 contiguous dims of D) of BOTH
batches — head-parallel over all 8 cores, batch handled inside each core.
This makes the output-projection exchange a single 8-core AllToAll per q-span
of the (normalized, bf16) attention outputs: shard j of core c's send buffer
is its yT slice for (batch j//4, q-tile j%4), and received slot i is D-chunk
i for the core's own (batch, q-tile) = (c//4, c%4). Every AP in that exchange
is core-independent, so one SPMD program serves all 8 cores, and the wire
traffic is ~1MB bf16 total instead of ReduceScattering 8MB of fp32 partials
per core. Each core then computes the full-D out-projection for its q-tile.

Device-side layout (host pre-transposes, pure data movement):
  - xT  [2, D, T]     = x[b].T so projections contract D on the partition dim.
  - qT/kT [b][128, T] computed directly transposed (dims on partitions);
                        the core's 2 heads at partitions 0-63 / 64-127.
  - scoresT[k, q]     = k @ qT; the two heads are computed by two row-tiled
                        matmuls (tile_position (0,0)/(64,0), K=64 each) that
                        run concurrently in the PE array, writing two
                        adjacent PSUM banks.
  - exp               one ScalarE activation per k-tile covers both heads'
                        scores ([128, 1024] across the 2 banks). Diagonal
                        tiles trim the leading fully-masked columns from the
                        scores matmul, the exp, and the AV matmul; the mask
                        values are applied only on the [128, 128] triangle
                        blocks.
  - v_aug [k, 2*65]   v with a ones column per head: AV yields yT' [65, span]
                        whose row 64 is the softmax denominator.
  - normalization     reciprocal of the denominator rows, broadcast across
                        partitions with one rank-33 selector matmul per
                        (span, batch), multiplied into yT in one DVE pass.
  - out-projection    after the AllToAll: 8 accumulating matmuls per
                        [128 q, 512] output tile (full-D contraction), bias
                        on DVE, DMA straight to the output.

Dtypes: all matmul operands bf16 with fp32 PSUM accumulation; exp and the
normalization run in fp32 (bf16 storage). ScalarE does nothing but exp; the
PE is kept warm with a short warm-up matmul burst and by interleaving
projection / out-projection matmuls between attention blocks.
"""

import os
import numpy as np
import ml_dtypes

BF16 = ml_dtypes.bfloat16

B, T, D, H = 2, 2048, 1024, 16
HD = D // H                     # 64
NCORES = 8
DL = D // NCORES                # dims per core = 128 (2 heads)
SP = 512                        # free-dim span per matmul (one PSUM bank, fp32)
QS = T // SP                    # 4 q spans
KT = T // 128                   # 16 k tiles
SCALE = HD ** -0.5

_CACHE = {}


def _build_program():
    import concourse.bass as bass  # noqa: F401  (registers bass machinery)
    import concourse.tile as tile
    from concourse import bacc, mybir

    f32 = mybir.dt.float32
    f32r = mybir.dt.float32r
    bf16 = mybir.dt.bfloat16
    Exp = mybir.ActivationFunctionType.Exp

    nc = bacc.Bacc("TRN2", target_bir_lowering=False, debug=False,
                   num_devices=NCORES)

    xT = nc.dram_tensor("xT", [B, D, T], bf16, kind="ExternalInput")
    wqT = nc.dram_tensor("wqT", [D, DL], bf16, kind="ExternalInput")
    wkT = nc.dram_tensor("wkT", [D, DL], bf16, kind="ExternalInput")
    wvT = nc.dram_tensor("wvT", [D, DL], bf16, kind="ExternalInput")
    woT = nc.dram_tensor("woT", [D, D], bf16, kind="ExternalInput")
    bqP = nc.dram_tensor("bqP", [128, 1], f32, kind="ExternalInput")
    bkP = nc.dram_tensor("bkP", [128, 1], f32, kind="ExternalInput")
    bv = nc.dram_tensor("bv", [1, DL], bf16, kind="ExternalInput")
    bo = nc.dram_tensor("bo", [1, D], bf16, kind="ExternalInput")
    mtriD = nc.dram_tensor("mtriD", [128, B * KT * 128], bf16,
                           kind="ExternalInput")
    out_ext = nc.dram_tensor("out", [QS, 128, D], f32, kind="ExternalOutput")

    RG = [[0, 1, 2, 3, 4, 5, 6, 7]]

    with tile.TileContext(nc) as tc:
        with tc.tile_pool(name="main", bufs=1) as main, \
             tc.tile_pool(name="dram", bufs=1, space="DRAM") as dram:
            xt_s = main.tile([128, B, 8, T], bf16)
            wq_s = main.tile([128, 8, DL], bf16)
            wk_s = main.tile([128, 8, DL], bf16)
            wv_s = main.tile([128, 8, DL], bf16)
            woT_s = main.tile([128, 8, D], bf16)
            qT_s = main.tile([128, B, T], bf16)
            kT_s = main.tile([128, B, T], bf16)
            yT_s = main.tile([128, B, T], bf16)
            v_s = main.tile([128, B, KT, 2 * 65], bf16)
            bq_s = main.tile([128, 1], f32)
            bk_s = main.tile([128, 1], f32)
            bv_bc = main.tile([128, DL], bf16)
            bo_bc = main.tile([128, D], bf16)
            mtri_s = main.tile([128, B, KT, 128], bf16)
            # selector for the denominator broadcast: rb = sel.T @ rec2
            # (rec2 rows 0/32 hold the two heads' 1/denominator; the other
            # rows are 1.0 and get selected by zeros)
            sel_s = main.tile([33, 128], bf16)
            rec_all = main.tile([33, B * QS, SP], bf16)
            recf_all = main.tile([33, B * QS, SP], f32)
            den_all = main.tile([33, B * QS, SP], f32)
            warm_s = main.tile([128, SP], bf16)
            dum_o = main.tile([1, 2], bf16)

            a2a_in = [dram.tile([NCORES * 128, 128], bf16, name=f"a2ai{s}")
                      for s in range(QS)]
            a2a_out = [dram.tile([NCORES * 128, 128], bf16, name=f"a2ao{s}")
                       for s in range(QS)]
            # constants (DVE) + ACT table warm-up before any real dependency
            nc.vector.memset(warm_s, 0.25)
            nc.vector.memset(v_s, 1.0)
            nc.vector.memset(sel_s, 0.0)
            nc.vector.memset(sel_s[0:1, 0:64], 1.0)
            nc.vector.memset(sel_s[32:33, 64:128], 1.0)
            nc.vector.memset(rec_all, 1.0)
            nc.vector.memset(recf_all, 1.0)
            nc.vector.memset(den_all, 1.0)
            nc.scalar.activation(dum_o, warm_s[0:1, 0:2], Exp)

            # loads: wq + batch-0 x first (the first projections need
            # them), mask triangles as one contiguous DMA, then the rest
            nc.sync.dma_start(out=bq_s, in_=bqP[:])
            nc.sync.dma_start(out=bk_s, in_=bkP[:])
            wq_r = wqT[:].rearrange("(c p) n -> c p n", p=128)
            for c in range(8):
                nc.sync.dma_start(out=wq_s[:, c, :], in_=wq_r[c])
            xT_r = xT[:].rearrange("b (c p) t -> b c p t", p=128)
            for c in range(8):
                eng = nc.sync if c % 2 == 0 else nc.gpsimd
                eng.dma_start(out=xt_s[:, 0, c, :], in_=xT_r[0, c])
            for w_s, w_d in ((wk_s, wkT), (wv_s, wvT)):
                w_r = w_d[:].rearrange("(c p) n -> c p n", p=128)
                for c in range(8):
                    nc.gpsimd.dma_start(out=w_s[:, c, :], in_=w_r[c])
            for c in range(8):
                eng = nc.sync if c % 2 == 0 else nc.gpsimd
                eng.dma_start(out=xt_s[:, 1, c, :], in_=xT_r[1, c])
            nc.sync.dma_start(
                out=mtri_s[:].rearrange("p b t q -> p (b t q)"),
                in_=mtriD[:])
            nc.gpsimd.dma_start(out=bv_bc, in_=bv[:].to_broadcast([128, DL]))
            nc.gpsimd.dma_start(out=bo_bc, in_=bo[:].to_broadcast([128, D]))
            woT_r = woT[:].rearrange("(c p) n -> c p n", p=128)
            for c in range(8):
                nc.gpsimd.dma_start(out=woT_s[:, c, :], in_=woT_r[c])

            with tc.tile_pool(name="sc_psum", bufs=2, space="PSUM") as sc_psum, \
                 tc.tile_pool(name="av_psum", bufs=1, space="PSUM") as av_psum, \
                 tc.tile_pool(name="mm_psum", bufs=2, space="PSUM") as mm_psum, \
                 tc.tile_pool(name="at_sb", bufs=6) as at_sb, \
                 tc.tile_pool(name="ytf_sb", bufs=2) as ytf_sb, \
                 tc.tile_pool(name="ob_sb", bufs=3) as ob_sb:

                # PE warm-up during the initial DMA wait: gets the HAM clock
                # gate to 8/8 before the first projection matmul
                for i in range(16):
                    wm = mm_psum.tile([128, SP], f32, tag="mm")
                    nc.tensor.matmul(wm, lhsT=warm_s[:, 0:128], rhs=warm_s,
                                     start=True, stop=True)

                def proj_block(sp):
                    # q/k for span sp and v for k-tiles 4sp..4sp+3, per batch
                    for b in range(B):
                        for w_s, b_s, dst in ((wq_s, bq_s, qT_s),
                                              (wk_s, bk_s, kT_s)):
                            ps = mm_psum.tile([128, SP], f32, tag="mm")
                            for kc in range(8):
                                nc.tensor.matmul(
                                    ps,
                                    lhsT=w_s[:, kc, :],
                                    rhs=xt_s[:, b, kc, sp * SP:(sp + 1) * SP],
                                    start=(kc == 0), stop=(kc == 7))
                            nc.vector.tensor_scalar_add(
                                dst[:, b, sp * SP:(sp + 1) * SP], ps, b_s)
                        for mt in range(4 * sp, 4 * sp + 4):
                            ps = mm_psum.tile([128, SP], f32, tag="mm")
                            for kc in range(8):
                                nc.tensor.matmul(
                                    ps[:, 0:DL],
                                    lhsT=xt_s[:, b, kc,
                                              mt * 128:(mt + 1) * 128],
                                    rhs=wv_s[:, kc, :],
                                    start=(kc == 0), stop=(kc == 7))
                            nc.vector.tensor_add(
                                v_s[:, b, mt, :].rearrange(
                                    "p (h d) -> p h d", d=65)[:, :, 0:64],
                                ps[:, 0:DL].rearrange(
                                    "p (h d) -> p h d", d=64),
                                bv_bc.rearrange("p (h d) -> p h d", d=64))

                def attn(sp, b):
                    # both heads for batch b; returns the rec slot
                    nkt = 4 * sp + 4
                    av = av_psum.tile([65, 2 * SP], f32, tag="av")

                    def sc_exp(kt):
                        # scores (row-tiled pair) + exp + triangle mask
                        c0 = max(0, 128 * (kt - 4 * sp))
                        sc = sc_psum.tile([128, 2 * SP], f32, tag="sc")
                        for hh in range(2):
                            r0 = 64 * hh
                            nc.tensor.matmul(
                                sc[:, hh * SP + c0:(hh + 1) * SP],
                                lhsT=kT_s[r0:r0 + 64, b,
                                          kt * 128:(kt + 1) * 128],
                                rhs=qT_s[r0:r0 + 64, b,
                                         sp * SP + c0:(sp + 1) * SP],
                                start=True, stop=True)
                        at = at_sb.tile([128, 2 * SP], bf16, tag="at")
                        if c0:
                            nc.scalar.activation(
                                at.rearrange("p (g q) -> p g q",
                                             g=2)[:, :, c0:],
                                sc.rearrange("p (g q) -> p g q",
                                             g=2)[:, :, c0:],
                                Exp)
                        else:
                            nc.scalar.activation(at, sc, Exp)
                        if kt >= 4 * sp:  # diagonal tile: mask the triangle
                            for hh in range(2):
                                blk = at[:, hh * SP + c0:hh * SP + c0 + 128]
                                nc.vector.tensor_mul(blk, blk,
                                                     mtri_s[:, b, kt, :])
                        return at, c0

                    # software-pipelined: the k-tile after next's scores are
                    # already in the PE queue when an AV waits on its exp
                    pend = {0: sc_exp(0)}
                    for kt in range(nkt):
                        if kt + 1 < nkt:
                            pend[kt + 1] = sc_exp(kt + 1)
                        at, c0 = pend.pop(kt)
                        for hh in range(2):
                            nc.tensor.matmul(
                                av[:, hh * SP + c0:(hh + 1) * SP],
                                lhsT=v_s[:, b, kt, hh * 65:(hh + 1) * 65],
                                rhs=at[:, hh * SP + c0:(hh + 1) * SP],
                                start=(kt == 0), stop=(kt == nkt - 1))
                    rec2 = rec_all[:, B * sp + b, :]
                    recf = recf_all[:, B * sp + b, :]
                    den2 = den_all[:, B * sp + b, :]
                    nc.vector.tensor_copy(den2[0:1, :], av[64:65, 0:SP])
                    nc.vector.tensor_copy(den2[32:33, :], av[64:65, SP:2 * SP])
                    nc.vector.reciprocal_approx_fast(out=recf, in_=den2)
                    nc.vector.tensor_copy(rec2, recf)
                    nc.vector.tensor_copy(yT_s[0:64, b, sp * SP:(sp + 1) * SP],
                                          av[0:64, 0:SP])
                    nc.vector.tensor_copy(yT_s[64:128, b,
                                               sp * SP:(sp + 1) * SP],
                                          av[0:64, SP:2 * SP])
                    return rec2

                def post(sp, b, rec2):
                    # broadcast 1/denominator across partitions via one
                    # rank-33 selector matmul, then normalize yT in place
                    rb = mm_psum.tile([128, SP], f32, tag="mm")
                    nc.tensor.matmul(rb, lhsT=sel_s, rhs=rec2,
                                     start=True, stop=True)
                    yv = yT_s[:, b, sp * SP:(sp + 1) * SP]
                    nc.vector.tensor_mul(yv, yv, rb)

                def exchange(sp):
                    # shard j = my yT slice for (batch j//4, q-tile j%4);
                    # slot i of the output = D-chunk i of my own q-tile
                    for b in range(B):
                        for t in range(QS):
                            j = QS * b + t
                            nc.sync.dma_start(
                                out=a2a_in[sp][j * 128:(j + 1) * 128, :],
                                in_=yT_s[:, b, sp * SP + t * 128:
                                         sp * SP + (t + 1) * 128])
                    nc.gpsimd.collective_compute(
                        "AllToAll", mybir.AluOpType.bypass,
                        replica_groups=RG,
                        ins=[a2a_in[sp][:].opt()],
                        outs=[a2a_out[sp][:].opt()])
                    ytf = ytf_sb.tile([128, 8, 128], bf16, tag="ytf")
                    nc.sync.dma_start(
                        out=ytf,
                        in_=a2a_out[sp][:].rearrange("(i p) q -> p i q",
                                                     p=128))
                    return ytf

                def outproj(sp, ytf):
                    # full-D out-projection for this core's q-tile of span sp
                    for ns in range(2):
                        po = mm_psum.tile([128, SP], f32, tag="mm")
                        for i in range(8):
                            nc.tensor.matmul(
                                po,
                                lhsT=ytf[:, i, :],
                                rhs=woT_s[:, i, ns * SP:(ns + 1) * SP],
                                start=(i == 0), stop=(i == 7))
                        ob = ob_sb.tile([128, SP], f32, tag="ob")
                        nc.vector.tensor_add(ob, po,
                                             bo_bc[:, ns * SP:(ns + 1) * SP])
                        nc.sync.dma_start(
                            out=out_ext[sp, :, ns * SP:(ns + 1) * SP], in_=ob)

                # software pipeline: post()/exchange()/outproj() are issued
                # behind later attention blocks so their PE work (which waits
                # on DVE/collective results) never stalls the PE queue
                recs = {}
                ytfs = {}
                proj_block(0)
                recs[(0, 0)] = attn(0, 0)
                proj_block(1)
                recs[(0, 1)] = attn(0, 1)
                post(0, 0, recs[(0, 0)])
                proj_block(2)
                recs[(1, 0)] = attn(1, 0)
                post(0, 1, recs[(0, 1)])
                ytfs[0] = exchange(0)
                proj_block(3)
                recs[(1, 1)] = attn(1, 1)
                post(1, 0, recs[(1, 0)])
                recs[(2, 0)] = attn(2, 0)
                post(1, 1, recs[(1, 1)])
                ytfs[1] = exchange(1)
                recs[(2, 1)] = attn(2, 1)
                post(2, 0, recs[(2, 0)])
                recs[(3, 0)] = attn(3, 0)
                post(2, 1, recs[(2, 1)])
                ytfs[2] = exchange(2)
                recs[(3, 1)] = attn(3, 1)
                post(3, 0, recs[(3, 0)])
                post(3, 1, recs[(3, 1)])
                ytfs[3] = exchange(3)
                # all out-projections strictly last: the PE FIFO must never
                # park behind an in-flight collective
                for sp in range(QS):
                    outproj(sp, ytfs[sp])

    nc.compile()
    return nc


def _get_program():
    if "nc" not in _CACHE:
        _CACHE["nc"] = _build_program()
    return _CACHE["nc"]


def _make_in_maps(x, mask, Wq, bq, Wk, bk, Wv, bv, Wo, bo):
    x = np.asarray(x, np.float32)
    mask = np.asarray(mask, bool)
    Wq = np.asarray(Wq, np.float32)
    Wk = np.asarray(Wk, np.float32)
    Wv = np.asarray(Wv, np.float32)
    Wo = np.asarray(Wo, np.float32)
    bq = np.asarray(bq, np.float32)
    bk = np.asarray(bk, np.float32)
    bv = np.asarray(bv, np.float32)
    bo = np.asarray(bo, np.float32)

    xTd = np.ascontiguousarray(x.transpose(0, 2, 1)).astype(BF16)  # [B, D, T]
    woT = np.ascontiguousarray(Wo.T).astype(BF16)
    bo_row = bo.reshape(1, D).astype(BF16)
    # the 16 diagonal [128,128] blocks of mask[b,0].T (k on rows),
    # partition-major so the load is one contiguous DMA
    md = np.empty((B, KT, 128, 128), np.float32)
    for b in range(B):
        mT = mask[b, 0].T
        for t in range(KT):
            md[b, t] = mT[t * 128:(t + 1) * 128, t * 128:(t + 1) * 128]
    md = np.ascontiguousarray(
        md.transpose(2, 0, 1, 3)).reshape(128, B * KT * 128).astype(BF16)

    in_maps = []
    for c in range(NCORES):
        sl = slice(c * DL, (c + 1) * DL)  # dims of heads {2c, 2c+1}
        in_maps.append({
            "xT": xTd,
            "wqT": np.ascontiguousarray((Wq[sl] * SCALE).T).astype(BF16),
            "wkT": np.ascontiguousarray(Wk[sl].T).astype(BF16),
            "wvT": np.ascontiguousarray(Wv[sl].T).astype(BF16),
            "woT": woT,
            "bqP": np.ascontiguousarray((bq[sl] * SCALE).reshape(DL, 1)),
            "bkP": np.ascontiguousarray(bk[sl].reshape(DL, 1)),
            "bv": bv[sl].reshape(1, DL).astype(BF16),
            "bo": bo_row,
            "mtriD": md,
        })
    return in_maps


def _capture_profile(nc, in_maps, tmpdir):
    """Run with NTFF capture and process the profile ourselves (the stock
    trace path can't handle the duplicate-executable NTFFs the axon relay
    produces). Returns (results, exec_time_ns|None)."""
    import glob
    import json
    import re
    import subprocess
    from trn_agent_boot.trn_boot import _ntff_profile_via_ctypes
    from concourse import bass2jax

    hook = _ntff_profile_via_ctypes("/opt/axon/libaxon_pjrt.so")
    if hook is None:
        raise RuntimeError("libaxon_pjrt.so lacks NTFF profile symbols")
    os.makedirs(tmpdir, exist_ok=True)
    with hook(tmpdir, [0]):
        results = bass2jax.run_bass_via_pjrt(nc, in_maps, n_cores=NCORES)

    # group NTFF/NEFF pairs by executable id; use the newest executable
    ntffs = glob.glob(os.path.join(tmpdir, "*_body*-device*.ntff"))
    best, best_id = None, -1
    for f in ntffs:
        m = re.search(r"executable(\d+)-device000000", f)
        if m and int(m.group(1)) > best_id:
            best_id, best = int(m.group(1)), f
    if best is None:
        raise RuntimeError(f"no NTFF produced in {tmpdir}")
    neff = re.sub(r"-device\d+-execution-\d+\.ntff$", ".neff", best)
    out_json = os.path.join(tmpdir, "prof.json")
    subprocess.check_call(
        ["neuron-profile", "view", "--ignore-nc-buf-usage", "-s", best,
         "-n", neff, "--output-format=json", f"--output-file={out_json}"],
        cwd=tmpdir)
    summary = json.load(open(out_json))["summary"][0]
    return results, int(summary["total_time"] * 1e9)


def kernel(x, mask, Wq, bq, Wk, bk, Wv, bv, Wo, bo):
    from concourse import bass_utils

    in_maps = _make_in_maps(x, mask, Wq, bq, Wk, bk, Wv, bv, Wo, bo)
    nc = _get_program()

    trace = bool(int(os.environ.get("MHA_TRACE", "0")))
    tmpdir = os.environ.get("MHA_TRACE_DIR") or None
    results = None
    if trace and tmpdir:
        try:
            results, exec_ns = _capture_profile(nc, in_maps, tmpdir)
            _CACHE["last_exec_time_ns"] = exec_ns
        except Exception as e:  # profiling is best-effort
            print(f"profiling unavailable: {type(e).__name__}: {e}")
            results = None
    if results is None:
        results = bass_utils.run_bass_kernel_spmd(
            nc, in_maps, core_ids=list(range(NCORES))).results
        _CACHE.setdefault("last_exec_time_ns", None)

    out = np.empty((B, T, D), np.float32)
    for c in range(NCORES):
        b, t = divmod(c, QS)  # core c owns (batch b, q-tile t) of every span
        o = results[c]["out"]
        for sp in range(QS):
            lo = sp * SP + t * 128
            out[b, lo:lo + 128] = o[sp]
    return out


# revision 20
# speedup vs baseline: 1.1594x; 1.0330x over previous
"""Causal multi-head attention (B=2, T=2048, D=1024, H=16) on 8 TRN2 NeuronCores.

Sharding: core c owns heads {2c, 2c+1} (= 128 contiguous dims of D) of BOTH
batches — head-parallel over all 8 cores, batch handled inside each core.
This makes the output-projection exchange a single 8-core AllToAll per q-span
of the (normalized, bf16) attention outputs: shard j of core c's send buffer
is its yT slice for (batch j//4, q-tile j%4), and received slot i is D-chunk
i for the core's own (batch, q-tile) = (c//4, c%4). Every AP in that exchange
is core-independent, so one SPMD program serves all 8 cores, and the wire
traffic is ~1MB bf16 total instead of ReduceScattering 8MB of fp32 partials
per core. Each core then computes the full-D out-projection for its q-tile.

Device-side layout (host pre-transposes, pure data movement):
  - xT  [2, D, T]     = x[b].T so projections contract D on the partition dim.
  - qT/kT [b][128, T] computed directly transposed (dims on partitions);
                        the core's 2 heads at partitions 0-63 / 64-127.
  - scoresT[k, q]     = k @ qT; the two heads are computed by two row-tiled
                        matmuls (tile_position (0,0)/(64,0), K=64 each) that
                        run concurrently in the PE array, writing two
                        adjacent PSUM banks.
  - exp               one ScalarE activation per k-tile covers both heads'
                        scores ([128, 1024] across the 2 banks). Diagonal
                        tiles trim the leading fully-masked columns from the
                        scores matmul, the exp, and the AV matmul; the mask
                        values are applied only on the [128, 128] triangle
                        blocks.
  - v_aug [k, 2*65]   v with a ones column per head: AV yields yT' [65, span]
                        whose row 64 is the softmax denominator.
  - normalization     reciprocal of the denominator rows, broadcast across
                        partitions with one rank-33 selector matmul per
                        (span, batch), multiplied into yT in one DVE pass.
  - out-projection    after the AllToAll: 8 accumulating matmuls per
                        [128 q, 512] output tile (full-D contraction), bias
                        on DVE, DMA straight to the output.

Dtypes: all matmul operands bf16 with fp32 PSUM accumulation; exp and the
normalization run in fp32 (bf16 storage). ScalarE does nothing but exp; the
PE is kept warm with a short warm-up matmul burst and by interleaving
projection / out-projection matmuls between attention blocks.
"""

import os
import numpy as np
import ml_dtypes

BF16 = ml_dtypes.bfloat16

B, T, D, H = 2, 2048, 1024, 16
HD = D // H                     # 64
NCORES = 8
DL = D // NCORES                # dims per core = 128 (2 heads)
SP = 512                        # free-dim span per matmul (one PSUM bank, fp32)
QS = T // SP                    # 4 q spans
KT = T // 128                   # 16 k tiles
SCALE = HD ** -0.5

_CACHE = {}


def _build_program():
    import concourse.bass as bass  # noqa: F401  (registers bass machinery)
    import concourse.tile as tile
    from concourse import bacc, mybir

    f32 = mybir.dt.float32
    f32r = mybir.dt.float32r
    bf16 = mybir.dt.bfloat16
    Exp = mybir.ActivationFunctionType.Exp

    nc = bacc.Bacc("TRN2", target_bir_lowering=False, debug=False,
                   num_devices=NCORES)

    xT = nc.dram_tensor("xT", [B, D, T], bf16, kind="ExternalInput")
    wqT = nc.dram_tensor("wqT", [D, DL], bf16, kind="ExternalInput")
    wkT = nc.dram_tensor("wkT", [D, DL], bf16, kind="ExternalInput")
    wvT = nc.dram_tensor("wvT", [D, DL], bf16, kind="ExternalInput")
    woT = nc.dram_tensor("woT", [D, D], bf16, kind="ExternalInput")
    bqP = nc.dram_tensor("bqP", [128, 1], f32, kind="ExternalInput")
    bkP = nc.dram_tensor("bkP", [128, 1], f32, kind="ExternalInput")
    bv = nc.dram_tensor("bv", [1, DL], bf16, kind="ExternalInput")
    bo = nc.dram_tensor("bo", [1, D], bf16, kind="ExternalInput")
    mtriD = nc.dram_tensor("mtriD", [128, B * KT * 128], bf16,
                           kind="ExternalInput")
    out_ext = nc.dram_tensor("out", [QS, 128, D], f32, kind="ExternalOutput")

    RG = [[0, 1, 2, 3, 4, 5, 6, 7]]

    with tile.TileContext(nc) as tc:
        with tc.tile_pool(name="main", bufs=1) as main, \
             tc.tile_pool(name="dram", bufs=1, space="DRAM") as dram:
            xt_s = main.tile([128, B, 8, T], bf16)
            wq_s = main.tile([128, 8, DL], bf16)
            wk_s = main.tile([128, 8, DL], bf16)
            wv_s = main.tile([128, 8, DL], bf16)
            woT_s = main.tile([128, 8, D], bf16)
            qT_s = main.tile([128, B, T], bf16)
            kT_s = main.tile([128, B, T], bf16)
            yT_s = main.tile([128, B, T], bf16)
            v_s = main.tile([128, B, KT, 2 * 65], bf16)
            bq_s = main.tile([128, 1], f32)
            bk_s = main.tile([128, 1], f32)
            bv_bc = main.tile([128, DL], bf16)
            bo_bc = main.tile([128, D], bf16)
            mtri_s = main.tile([128, B, KT, 128], bf16)
            # selector for the denominator broadcast: rb = sel.T @ rec2
            # (rec2 rows 0/32 hold the two heads' 1/denominator; the other
            # rows are 1.0 and get selected by zeros)
            sel_s = main.tile([33, 128], bf16)
            rec_all = main.tile([33, B * QS, SP], bf16)
            recf_all = main.tile([33, B * QS, SP], f32)
            den_all = main.tile([33, B * QS, SP], f32)
            warm_s = main.tile([128, SP], bf16)
            dum_o = main.tile([1, 2], bf16)

            a2a_in = [dram.tile([NCORES * 128, 128], bf16, name=f"a2ai{s}")
                      for s in range(QS)]
            a2a_out = [dram.tile([NCORES * 128, 128], bf16, name=f"a2ao{s}")
                       for s in range(QS)]
            # constants (DVE) + ACT table warm-up before any real dependency
            nc.vector.memset(warm_s, 0.25)
            nc.vector.memset(v_s, 1.0)
            nc.vector.memset(sel_s, 0.0)
            nc.vector.memset(sel_s[0:1, 0:64], 1.0)
            nc.vector.memset(sel_s[32:33, 64:128], 1.0)
            nc.vector.memset(rec_all, 1.0)
            nc.vector.memset(recf_all, 1.0)
            nc.vector.memset(den_all, 1.0)
            nc.scalar.activation(dum_o, warm_s[0:1, 0:2], Exp)

            # loads spread over four engine DMA queues; batch-0 x + the
            # qkv weights land first, batch-1 x next, bulk (woT) last
            engs = [nc.sync, nc.gpsimd, nc.scalar]
            nc.sync.dma_start(out=bq_s, in_=bqP[:])
            nc.sync.dma_start(out=bk_s, in_=bkP[:])
            wq_r = wqT[:].rearrange("(c p) n -> c p n", p=128)
            for c in range(8):
                nc.sync.dma_start(out=wq_s[:, c, :], in_=wq_r[c])
            xT_r = xT[:].rearrange("b (c p) t -> b c p t", p=128)
            for c in range(8):
                engs[c % 3].dma_start(out=xt_s[:, 0, c, :], in_=xT_r[0, c])
            for w_s, w_d in ((wk_s, wkT), (wv_s, wvT)):
                w_r = w_d[:].rearrange("(c p) n -> c p n", p=128)
                for c in range(8):
                    nc.gpsimd.dma_start(out=w_s[:, c, :], in_=w_r[c])
            nc.scalar.dma_start(
                out=mtri_s[:].rearrange("p b t q -> p (b t q)"),
                in_=mtriD[:])
            for c in range(8):
                engs[c % 3].dma_start(out=xt_s[:, 1, c, :], in_=xT_r[1, c])
            nc.scalar.dma_start(out=bv_bc, in_=bv[:].to_broadcast([128, DL]))
            nc.scalar.dma_start(out=bo_bc, in_=bo[:].to_broadcast([128, D]))
            woT_r = woT[:].rearrange("(c p) n -> c p n", p=128)
            for c in range(8):
                engs[c % 3].dma_start(out=woT_s[:, c, :], in_=woT_r[c])

            with tc.tile_pool(name="sc_psum", bufs=2, space="PSUM") as sc_psum, \
                 tc.tile_pool(name="av_psum", bufs=1, space="PSUM") as av_psum, \
                 tc.tile_pool(name="mm_psum", bufs=2, space="PSUM") as mm_psum, \
                 tc.tile_pool(name="at_sb", bufs=6) as at_sb, \
                 tc.tile_pool(name="ytf_sb", bufs=2) as ytf_sb, \
                 tc.tile_pool(name="ob_sb", bufs=3) as ob_sb:

                # PE warm-up during the initial DMA wait: gets the HAM clock
                # gate to 8/8 before the first projection matmul
                for i in range(16):
                    wm = mm_psum.tile([128, SP], f32, tag="mm")
                    nc.tensor.matmul(wm, lhsT=warm_s[:, 0:128], rhs=warm_s,
                                     start=True, stop=True)

                def proj_block(sp):
                    # q/k for span sp and v for k-tiles 4sp..4sp+3, per batch
                    for b in range(B):
                        for w_s, b_s, dst in ((wq_s, bq_s, qT_s),
                                              (wk_s, bk_s, kT_s)):
                            ps = mm_psum.tile([128, SP], f32, tag="mm")
                            for kc in range(8):
                                nc.tensor.matmul(
                                    ps,
                                    lhsT=w_s[:, kc, :],
                                    rhs=xt_s[:, b, kc, sp * SP:(sp + 1) * SP],
                                    start=(kc == 0), stop=(kc == 7))
                            nc.vector.tensor_scalar_add(
                                dst[:, b, sp * SP:(sp + 1) * SP], ps, b_s)
                        for mt in range(4 * sp, 4 * sp + 4):
                            ps = mm_psum.tile([128, SP], f32, tag="mm")
                            for kc in range(8):
                                nc.tensor.matmul(
                                    ps[:, 0:DL],
                                    lhsT=xt_s[:, b, kc,
                                              mt * 128:(mt + 1) * 128],
                                    rhs=wv_s[:, kc, :],
                                    start=(kc == 0), stop=(kc == 7))
                            nc.vector.tensor_add(
                                v_s[:, b, mt, :].rearrange(
                                    "p (h d) -> p h d", d=65)[:, :, 0:64],
                                ps[:, 0:DL].rearrange(
                                    "p (h d) -> p h d", d=64),
                                bv_bc.rearrange("p (h d) -> p h d", d=64))

                def attn(sp, b):
                    # both heads for batch b; returns the rec slot
                    nkt = 4 * sp + 4
                    av = av_psum.tile([65, 2 * SP], f32, tag="av")

                    def sc_exp(kt):
                        # scores (row-tiled pair) + exp + triangle mask
                        c0 = max(0, 128 * (kt - 4 * sp))
                        sc = sc_psum.tile([128, 2 * SP], f32, tag="sc")
                        for hh in range(2):
                            r0 = 64 * hh
                            nc.tensor.matmul(
                                sc[:, hh * SP + c0:(hh + 1) * SP],
                                lhsT=kT_s[r0:r0 + 64, b,
                                          kt * 128:(kt + 1) * 128],
                                rhs=qT_s[r0:r0 + 64, b,
                                         sp * SP + c0:(sp + 1) * SP],
                                start=True, stop=True)
                        at = at_sb.tile([128, 2 * SP], bf16, tag="at")
                        if c0:
                            nc.scalar.activation(
                                at.rearrange("p (g q) -> p g q",
                                             g=2)[:, :, c0:],
                                sc.rearrange("p (g q) -> p g q",
                                             g=2)[:, :, c0:],
                                Exp)
                        else:
                            nc.scalar.activation(at, sc, Exp)
                        if kt >= 4 * sp:  # diagonal tile: mask the triangle
                            for hh in range(2):
                                blk = at[:, hh * SP + c0:hh * SP + c0 + 128]
                                nc.vector.tensor_mul(blk, blk,
                                                     mtri_s[:, b, kt, :])
                        return at, c0

                    # software-pipelined: the k-tile after next's scores are
                    # already in the PE queue when an AV waits on its exp
                    pend = {0: sc_exp(0)}
                    for kt in range(nkt):
                        if kt + 1 < nkt:
                            pend[kt + 1] = sc_exp(kt + 1)
                        at, c0 = pend.pop(kt)
                        for hh in range(2):
                            nc.tensor.matmul(
                                av[:, hh * SP + c0:(hh + 1) * SP],
                                lhsT=v_s[:, b, kt, hh * 65:(hh + 1) * 65],
                                rhs=at[:, hh * SP + c0:(hh + 1) * SP],
                                start=(kt == 0), stop=(kt == nkt - 1))
                    rec2 = rec_all[:, B * sp + b, :]
                    recf = recf_all[:, B * sp + b, :]
                    den2 = den_all[:, B * sp + b, :]
                    nc.vector.tensor_copy(den2[0:1, :], av[64:65, 0:SP])
                    nc.vector.tensor_copy(den2[32:33, :], av[64:65, SP:2 * SP])
                    nc.vector.reciprocal_approx_fast(out=recf, in_=den2)
                    nc.vector.tensor_copy(rec2, recf)
                    nc.vector.tensor_copy(yT_s[0:64, b, sp * SP:(sp + 1) * SP],
                                          av[0:64, 0:SP])
                    nc.vector.tensor_copy(yT_s[64:128, b,
                                               sp * SP:(sp + 1) * SP],
                                          av[0:64, SP:2 * SP])
                    return rec2

                def post(sp, b, rec2):
                    # broadcast 1/denominator across partitions via one
                    # rank-33 selector matmul, then normalize yT in place
                    rb = mm_psum.tile([128, SP], f32, tag="mm")
                    nc.tensor.matmul(rb, lhsT=sel_s, rhs=rec2,
                                     start=True, stop=True)
                    yv = yT_s[:, b, sp * SP:(sp + 1) * SP]
                    nc.vector.tensor_mul(yv, yv, rb)

                def exchange(sp):
                    # shard j = my yT slice for (batch j//4, q-tile j%4);
                    # slot i of the output = D-chunk i of my own q-tile
                    for b in range(B):
                        for t in range(QS):
                            j = QS * b + t
                            nc.sync.dma_start(
                                out=a2a_in[sp][j * 128:(j + 1) * 128, :],
                                in_=yT_s[:, b, sp * SP + t * 128:
                                         sp * SP + (t + 1) * 128])
                    nc.gpsimd.collective_compute(
                        "AllToAll", mybir.AluOpType.bypass,
                        replica_groups=RG,
                        ins=[a2a_in[sp][:].opt()],
                        outs=[a2a_out[sp][:].opt()])
                    ytf = ytf_sb.tile([128, 8, 128], bf16, tag="ytf")
                    nc.sync.dma_start(
                        out=ytf,
                        in_=a2a_out[sp][:].rearrange("(i p) q -> p i q",
                                                     p=128))
                    return ytf

                def outproj(sp, ytf):
                    # full-D out-projection for this core's q-tile of span sp
                    for ns in range(2):
                        po = mm_psum.tile([128, SP], f32, tag="mm")
                        for i in range(8):
                            nc.tensor.matmul(
                                po,
                                lhsT=ytf[:, i, :],
                                rhs=woT_s[:, i, ns * SP:(ns + 1) * SP],
                                start=(i == 0), stop=(i == 7))
                        ob = ob_sb.tile([128, SP], f32, tag="ob")
                        nc.vector.tensor_add(ob, po,
                                             bo_bc[:, ns * SP:(ns + 1) * SP])
                        nc.sync.dma_start(
                            out=out_ext[sp, :, ns * SP:(ns + 1) * SP], in_=ob)

                # software pipeline: post()/exchange()/outproj() are issued
                # behind later attention blocks so their PE work (which waits
                # on DVE/collective results) never stalls the PE queue
                recs = {}
                ytfs = {}
                proj_block(0)
                recs[(0, 0)] = attn(0, 0)
                proj_block(1)
                recs[(0, 1)] = attn(0, 1)
                post(0, 0, recs[(0, 0)])
                proj_block(2)
                recs[(1, 0)] = attn(1, 0)
                post(0, 1, recs[(0, 1)])
                ytfs[0] = exchange(0)
                proj_block(3)
                recs[(1, 1)] = attn(1, 1)
                post(1, 0, recs[(1, 0)])
                recs[(2, 0)] = attn(2, 0)
                post(1, 1, recs[(1, 1)])
                ytfs[1] = exchange(1)
                recs[(2, 1)] = attn(2, 1)
                post(2, 0, recs[(2, 0)])
                recs[(3, 0)] = attn(3, 0)
                post(2, 1, recs[(2, 1)])
                ytfs[2] = exchange(2)
                recs[(3, 1)] = attn(3, 1)
                post(3, 0, recs[(3, 0)])
                post(3, 1, recs[(3, 1)])
                ytfs[3] = exchange(3)
                # all out-projections strictly last: the PE FIFO must never
                # park behind an in-flight collective
                for sp in range(QS):
                    outproj(sp, ytfs[sp])

    nc.compile()
    return nc


def _get_program():
    if "nc" not in _CACHE:
        _CACHE["nc"] = _build_program()
    return _CACHE["nc"]


def _make_in_maps(x, mask, Wq, bq, Wk, bk, Wv, bv, Wo, bo):
    x = np.asarray(x, np.float32)
    mask = np.asarray(mask, bool)
    Wq = np.asarray(Wq, np.float32)
    Wk = np.asarray(Wk, np.float32)
    Wv = np.asarray(Wv, np.float32)
    Wo = np.asarray(Wo, np.float32)
    bq = np.asarray(bq, np.float32)
    bk = np.asarray(bk, np.float32)
    bv = np.asarray(bv, np.float32)
    bo = np.asarray(bo, np.float32)

    xTd = np.ascontiguousarray(x.transpose(0, 2, 1)).astype(BF16)  # [B, D, T]
    woT = np.ascontiguousarray(Wo.T).astype(BF16)
    bo_row = bo.reshape(1, D).astype(BF16)
    # the 16 diagonal [128,128] blocks of mask[b,0].T (k on rows),
    # partition-major so the load is one contiguous DMA
    md = np.empty((B, KT, 128, 128), np.float32)
    for b in range(B):
        mT = mask[b, 0].T
        for t in range(KT):
            md[b, t] = mT[t * 128:(t + 1) * 128, t * 128:(t + 1) * 128]
    md = np.ascontiguousarray(
        md.transpose(2, 0, 1, 3)).reshape(128, B * KT * 128).astype(BF16)

    in_maps = []
    for c in range(NCORES):
        sl = slice(c * DL, (c + 1) * DL)  # dims of heads {2c, 2c+1}
        in_maps.append({
            "xT": xTd,
            "wqT": np.ascontiguousarray((Wq[sl] * SCALE).T).astype(BF16),
            "wkT": np.ascontiguousarray(Wk[sl].T).astype(BF16),
            "wvT": np.ascontiguousarray(Wv[sl].T).astype(BF16),
            "woT": woT,
            "bqP": np.ascontiguousarray((bq[sl] * SCALE).reshape(DL, 1)),
            "bkP": np.ascontiguousarray(bk[sl].reshape(DL, 1)),
            "bv": bv[sl].reshape(1, DL).astype(BF16),
            "bo": bo_row,
            "mtriD": md,
        })
    return in_maps


def _capture_profile(nc, in_maps, tmpdir):
    """Run with NTFF capture and process the profile ourselves (the stock
    trace path can't handle the duplicate-executable NTFFs the axon relay
    produces). Returns (results, exec_time_ns|None)."""
    import glob
    import json
    import re
    import subprocess
    from trn_agent_boot.trn_boot import _ntff_profile_via_ctypes
    from concourse import bass2jax

    hook = _ntff_profile_via_ctypes("/opt/axon/libaxon_pjrt.so")
    if hook is None:
        raise RuntimeError("libaxon_pjrt.so lacks NTFF profile symbols")
    os.makedirs(tmpdir, exist_ok=True)
    with hook(tmpdir, [0]):
        results = bass2jax.run_bass_via_pjrt(nc, in_maps, n_cores=NCORES)

    # group NTFF/NEFF pairs by executable id; use the newest executable
    ntffs = glob.glob(os.path.join(tmpdir, "*_body*-device*.ntff"))
    best, best_id = None, -1
    for f in ntffs:
        m = re.search(r"executable(\d+)-device000000", f)
        if m and int(m.group(1)) > best_id:
            best_id, best = int(m.group(1)), f
    if best is None:
        raise RuntimeError(f"no NTFF produced in {tmpdir}")
    neff = re.sub(r"-device\d+-execution-\d+\.ntff$", ".neff", best)
    out_json = os.path.join(tmpdir, "prof.json")
    subprocess.check_call(
        ["neuron-profile", "view", "--ignore-nc-buf-usage", "-s", best,
         "-n", neff, "--output-format=json", f"--output-file={out_json}"],
        cwd=tmpdir)
    summary = json.load(open(out_json))["summary"][0]
    return results, int(summary["total_time"] * 1e9)


def kernel(x, mask, Wq, bq, Wk, bk, Wv, bv, Wo, bo):
    from concourse import bass_utils

    in_maps = _make_in_maps(x, mask, Wq, bq, Wk, bk, Wv, bv, Wo, bo)
    nc = _get_program()

    trace = bool(int(os.environ.get("MHA_TRACE", "0")))
    tmpdir = os.environ.get("MHA_TRACE_DIR") or None
    results = None
    if trace and tmpdir:
        try:
            results, exec_ns = _capture_profile(nc, in_maps, tmpdir)
            _CACHE["last_exec_time_ns"] = exec_ns
        except Exception as e:  # profiling is best-effort
            print(f"profiling unavailable: {type(e).__name__}: {e}")
            results = None
    if results is None:
        results = bass_utils.run_bass_kernel_spmd(
            nc, in_maps, core_ids=list(range(NCORES))).results
        _CACHE.setdefault("last_exec_time_ns", None)

    out = np.empty((B, T, D), np.float32)
    for c in range(NCORES):
        b, t = divmod(c, QS)  # core c owns (batch b, q-tile t) of every span
        o = results[c]["out"]
        for sp in range(QS):
            lo = sp * SP + t * 128
            out[b, lo:lo + 128] = o[sp]
    return out
